# revision 66
# baseline (speedup 1.0000x reference)
"""MultiHeadGAT Bass kernel for Trainium2 (8 NeuronCores, batch-parallel).

Math (per batch b, head h):
  Wh = x @ W[h]                      (N, F_OUT)
  s1_i = Wh @ a1, s2_j = Wh @ a2     (N,)
  z[i,j] = s1_i + s2_j + ab
  exps = exp(leaky_relu(z, 0.2)) * A
  attn[i,j] = exps[i,j] / (sum_i' exps[i',j] + eps)    (softmax over dim i!)
  out = attn @ Wh; concat heads

Key identities:
  exp(leaky(z)) = exp(0.2 z) * max(exp(0.8 z), 1) = max(exp(z), exp(0.2 z)).
  Per-column factors exp(c * s2_j) cancel in the column softmax, so each
  column strip may use its own normalization convention.  In transposed
  layout (j on partitions) three strip paths are used, balanced across
  engines:
   - DVE path: one custom DVE op per strip computes
       ET[j,i] = AT * e5r_i * max(e5r_i^4 * e4c_j, 1), d_j += sum_i ET
     with e5r_i = exp(0.2(s1_i+ab)), e4c_j = exp(0.8 s2_j)   (e5c canceled)
   - ACT path (zero-DVE): z built on PE in PSUM, leaky via Relu trick,
     masked exp with accumulate on ACT (uncanceled convention).
   - Pool path: two gpsimd scalar_tensor_tensor ops compute
       ET = AT * max(e1r_i * e4c_j, e5r_i)  and d (canceled convention).
  TensorE then computes outT[o,i] += (Wh[j,o]/d_j) . ET[j,i].
"""

import numpy as np
import ml_dtypes
from operator import add

import concourse.bass as bass
import concourse.bacc as bacc
import concourse.mybir as mybir
import concourse.tile as tile
import concourse.dve_ops as dve_ops_mod
from concourse.dve_spec import (Spec, Src0, Src1, C0, C1, C2, One, sq, maxx,
                                lower, _has_src1)
from concourse.dve_uop import DveOpSpec
from concourse.bass_utils import run_bass_kernel_spmd

B, N, F_IN, F_OUT, H = 8, 1024, 128, 64, 4
EPS = 1e-7
NEG_SLOPE = 0.2
NCORES = 8
NSTRIP = N // 128  # 8 j-strips per core
HF = H * F_OUT     # 256

F32 = mybir.dt.float32
BF16 = mybir.dt.bfloat16
nbf16 = ml_dtypes.bfloat16

# path per (h, js): 'd' = custom DVE, 'a' = ACT relu path, 'p' = Pool STT
PATHS = {
    0: "dddddddd",
    2: "pppppppp",
    1: "pppddddd",
    3: "aadddddd",
}
A_HEADS = sorted({h for h, s in PATHS.items() if "a" in s})
P_HEADS = sorted({h for h, s in PATHS.items() if "p" in s})


# --------------------------------------------------------------------------
# custom DVE op: ET = Src0 * Src1 * max(Src1^4 * C0, 1);  d += sum(ET), seed C2
#   Src0 = AT strip (mask, bf16), Src1 = e5r broadcast field (fp32)
#   C0 = e4c per-partition, imm2 = EPS accum seed
# --------------------------------------------------------------------------
def _gat2_ref(in0, in1, c0, c1, c2):
    a = np.asarray(in0, np.float32)
    e5r = np.asarray(in1, np.float32)
    P = a.shape[0]
    e4c = np.broadcast_to(np.asarray(c0, np.float32).reshape(-1, 1), (P, 1))
    m = np.maximum((e5r ** 4) * e4c, np.float32(1.0))
    body = (a * e5r * m).astype(np.float32)
    return body, np.float32(c2) + body.reshape(P, -1).sum(axis=-1, keepdims=True)


def _register_gat2_op():
    name = "GAT_EXPS_MASK_REDUCE2"
    for o in dve_ops_mod.OPS:
        if o.name == name:
            return o
    m = maxx(sq(sq(Src1)) * C0, One)
    spec = Spec(body=Src0 * Src1 * m, accum=add, accum_init=C2,
                reference=_gat2_ref)
    shas = {}
    for ver in ("v3", "v4"):
        tmp = DveOpSpec(name=name, opcode=1, uops=lower(spec, ver=ver),
                        rd1_en=_has_src1(spec))
        shas[ver] = tmp.sha(ver)
    op = dve_ops_mod.DveOp(name, spec, False, shas)
    dve_ops_mod.OPS.append(op)
    dve_ops_mod.CUSTOM_DVE_SPECS[name] = spec
    dve_ops_mod._SUB_OPCODE_FOR_NAME[name] = (
        dve_ops_mod._CUSTOM_DVE_ROW_BASE + len(dve_ops_mod.OPS) - 1
    )
    assert dve_ops_mod._SUB_OPCODE_FOR_NAME[name] < 0x20
    return op


GAT2_OP = _register_gat2_op()


# --------------------------------------------------------------------------
# device program (SPMD; same program on all 8 cores, per-core data differs)
# --------------------------------------------------------------------------
def build_nc():
    nc = bacc.Bacc("TRN2", target_bir_lowering=False, debug=False,
                   enable_asserts=False, num_devices=NCORES)

    at_d = nc.dram_tensor("at", [N, N], BF16, kind="ExternalInput").ap()
    xt_d = nc.dram_tensor("xt", [F_IN, N], BF16, kind="ExternalInput").ap()
    # wcat: [W per head (256) | 0.8*W@a2 (4) | raw W@a2 (4)]
    wcat_d = nc.dram_tensor("wcat", [F_IN, HF + 2 * H], BF16,
                            kind="ExternalInput").ap()
    # w1s: cols 32h = 0.2*W@a1, cols 32h+16 = W@a1 (full scale)
    w1s_d = nc.dram_tensor("w1s", [F_IN, 113], BF16, kind="ExternalInput").ap()
    # bias column: 0.2*ab at partition 32h, ab at partition 32h+16
    bcol_d = nc.dram_tensor("bcol", [128, 1], F32, kind="ExternalInput").ap()
    # bias line at partition 0: [ab_h | 0.2*ab_h]
    bline_d = nc.dram_tensor("bline", [1, 2 * H], F32, kind="ExternalInput").ap()
    eye_d = nc.dram_tensor("eye", [128, 256], BF16, kind="ExternalInput").ap()
    ones1_d = nc.dram_tensor("ones1", [1, 128], BF16,
                             kind="ExternalInput").ap()
    ot_d = nc.dram_tensor("ot", [H, F_OUT, N], BF16,
                          kind="ExternalOutput").ap()

    M = mybir.AluOpType
    AF = mybir.ActivationFunctionType

    with tile.TileContext(nc) as tc:
        with (
            tc.tile_pool(name="const", bufs=1) as cpool,
            tc.tile_pool(name="whsb", bufs=NSTRIP) as whpool,
            tc.tile_pool(name="cols", bufs=NSTRIP) as colpool,
            tc.tile_pool(name="work", bufs=4) as wpool,
            tc.tile_pool(name="et", bufs=1) as etpool,
            tc.tile_pool(name="small", bufs=6) as spool,
            tc.tile_pool(name="ps1", bufs=1, space="PSUM") as ps1,
            tc.tile_pool(name="psw", bufs=2, space="PSUM") as psw,
            tc.tile_pool(name="psot", bufs=2, space="PSUM") as psot,
        ):
            # ---- phase 0: load inputs ------------------------------------
            xt = cpool.tile([F_IN, N], BF16, tag="xt")
            wcat = cpool.tile([F_IN, HF + 2 * H], BF16, tag="wcat")
            w1s = cpool.tile([F_IN, 113], BF16, tag="w1s")
            bcol = cpool.tile([128, 1], F32, tag="bcol")
            bline = cpool.tile([1, 2 * H], F32, tag="bline",
                               padded_shape=[128, 2 * H])
            # critical-path DMAs on SP in need-order; smalls on Pool queue
            nc.sync.dma_start(w1s[:], w1s_d[:])
            nc.sync.dma_start(xt[:, 0:512], xt_d[:, 0:512])
            nc.sync.dma_start(xt[:, 512:1024], xt_d[:, 512:1024])
            nc.sync.dma_start(wcat[:], wcat_d[:])
            nc.scalar.dma_start(bcol[:], bcol_d[:])
            nc.scalar.dma_start(bline[:], bline_d[:])
            eye = cpool.tile([128, 256], BF16, tag="eye")
            ones1 = cpool.tile([1, 128], BF16, tag="ones1",
                               padded_shape=[128, 128])
            if A_HEADS:
                nc.gpsimd.dma_start(eye[:], eye_d[:])
                nc.gpsimd.dma_start(ones1[:], ones1_d[:])

            at_sb = {}
            for js in range(NSTRIP):
                at_t = etpool.tile([128, N], BF16, tag=f"at{js}",
                                   name=f"at{js}")
                nc.sync.dma_start(at_t[:], at_d[js * 128:(js + 1) * 128, :])
                at_sb[js] = at_t

            # preload the Exp ACT table (overlaps input DMAs)
            warm = cpool.tile([1, 8], F32, tag="warm", padded_shape=[128, 8])
            nc.vector.memset(warm[:], 0.0)
            nc.scalar.activation(warm[0:1, :], warm[0:1, :], AF.Exp)
            warmb = cpool.tile([128, 8], F32, tag="warmb")
            nc.gpsimd.partition_broadcast(warmb[:], warm[0:1, :])


            # ---- phase 1: s-rows, exps, broadcasts -----------------------
            # srow_ps[32h]    = 0.2 * (W@a1_h)^T x   (0.2-scaled s1 rows)
            # srow_ps[32h+16] = full-scale s1 rows
            srow = ps1.tile([113, N], F32, tag="srow", name="srow")
            nc.tensor.matmul(srow[:, 0:512], w1s[:], xt[:, 0:512],
                             start=True, stop=True)
            nc.tensor.matmul(srow[:, 512:1024], w1s[:], xt[:, 512:1024],
                             start=True, stop=True)
            # one exp covers e5 rows (at 32h; bias 0.2 ab); rows at 32h+16
            # also get exp'd (bias ab) but are only consumed via the
            # per-head aligned tiles below.  Emitted in halves so the h0
            # broadcast (critical path) starts as early as possible.
            erows = cpool.tile([113, N], F32, tag="erows")
            for ns in (slice(0, 512), slice(512, 1024)):
                nc.scalar.activation(erows[:, ns], srow[:, ns], AF.Exp,
                                     bias=bcol[0:113, :], scale=1.0)

            # phase 2 bodies (emitted per-js; first two js before row ops so
            # the first DVE/Pool strips unblock early)
            wh_sb = [None] * NSTRIP
            e4c_sb = [None] * NSTRIP
            craw_sb = [None] * NSTRIP

            def emit_wh(js):
                whsc = psw.tile([128, HF + 2 * H], F32, tag="whsc",
                                name=f"whsc{js}")
                nc.tensor.matmul(whsc[:], xt[:, js * 128:(js + 1) * 128],
                                 wcat[:], start=True, stop=True)
                e4c = colpool.tile([128, H], F32, tag="e4c", name=f"e4c{js}")
                nc.scalar.activation(e4c[:], whsc[:, HF:HF + H], AF.Exp)
                e4c_sb[js] = e4c
                whb = whpool.tile([128, HF], BF16, tag="whb", name=f"whb{js}")
                nc.scalar.copy(whb[:], whsc[:, 0:HF])
                wh_sb[js] = whb
                if any(PATHS[h][js] == "a" for h in A_HEADS):
                    craw = colpool.tile([128, H], F32, tag="craw",
                                        name=f"craw{js}")
                    nc.vector.tensor_copy(craw[:], whsc[:, HF + H:HF + 2 * H])
                    craw_sb[js] = craw

            emit_wh(0)
            emit_wh(1)

            # srb rows (bf16 0.2(s1+ab)) for the ACT path, at partition 0
            srb = {}
            for h in A_HEADS:
                t = cpool.tile([1, N], BF16, tag=f"srb{h}", name=f"srb{h}",
                               padded_shape=[128, N])
                nc.scalar.activation(t[0:1, :], srow[32 * h:32 * h + 1, :],
                                     AF.Identity,
                                     bias=bline[0:1, H + h:H + h + 1],
                                     scale=1.0)
                srb[h] = t
            # e1 rows (fp32 exp(s1+ab)) for the Pool path, at partition 0
            e1row = {}
            for h in P_HEADS:
                t = cpool.tile([1, N], F32, tag=f"e1row{h}", name=f"e1row{h}",
                               padded_shape=[128, N])
                nc.scalar.activation(t[0:1, :],
                                     srow[32 * h + 16:32 * h + 17, :],
                                     AF.Exp, bias=bline[0:1, h:h + 1],
                                     scale=1.0)
                e1row[h] = t

            e5rb = [None] * H
            e1rb = [None] * H

            def emit_bcast(h):
                t = cpool.tile([128, N], F32, tag=f"e5rb{h}", name=f"e5rb{h}")
                if h == 0:
                    for ns in (slice(0, 512), slice(512, 1024)):
                        nc.gpsimd.partition_broadcast(
                            t[:, ns], erows[0:1, ns])
                else:
                    nc.gpsimd.partition_broadcast(
                        t[:], erows[32 * h:32 * h + 1, :])
                e5rb[h] = t
                if h in P_HEADS:
                    t2 = cpool.tile([128, N], F32, tag=f"e1rb{h}",
                                    name=f"e1rb{h}")
                    nc.gpsimd.partition_broadcast(t2[:], e1row[h][0:1, :])
                    e1rb[h] = t2

            # wave A heads broadcast now; wave B heads are interleaved into
            # the wave A loop (Pool's queue is FIFO — early broadcasts would
            # delay its first field ops).
            emit_bcast(0)
            emit_bcast(2)

            # ---- phase 2 (rest): Wh, e4c, whb ----------------------------
            for js in range(2, NSTRIP):
                emit_wh(js)

            # ---- phase 3: field compute, path-dependent ------------------
            ot_ps = [psot.tile([128, N], F32, tag="ot", name=f"otps{i}")
                     for i in range(2)]

            et_sb = {}
            whp_sb = {}

            def emit_field(h, js, path):
                et = etpool.tile([128, N], BF16, tag=f"et{h}_{js}",
                                 name=f"et{h}_{js}")
                ds = etpool.tile([128, 1], F32, tag=f"d{h}_{js}",
                                 name=f"d{h}_{js}")
                rs = etpool.tile([128, 1], F32, tag=f"r{h}_{js}",
                                 name=f"r{h}_{js}")
                if path == "d":
                    if (h, js) in ((0, 0), (0, 1)):
                        # ramp: process halves so the first strip starts
                        # right after the first broadcast half lands
                        ds0 = spool.tile([128, 1], F32, tag="ds0",
                                         name=f"ds0_{h}_{js}")
                        for k, ns in enumerate((slice(0, 512),
                                                slice(512, 1024))):
                            nc.vector._custom_dve(
                                GAT2_OP, out=et[:, ns],
                                in0=at_sb[js][:, ns], in1=e5rb[h][:, ns],
                                s0=e4c_sb[js][:, h:h + 1],
                                s1=0.0, imm2=EPS if k == 0 else 0.0,
                                accum_out=(ds0 if k == 0 else ds)[:])
                        nc.vector.tensor_tensor(out=ds[:], in0=ds[:],
                                                in1=ds0[:], op=M.add)
                    else:
                        nc.vector._custom_dve(
                            GAT2_OP, out=et[:], in0=at_sb[js][:],
                            in1=e5rb[h][:],
                            s0=e4c_sb[js][:, h:h + 1],
                            s1=0.0, imm2=EPS, accum_out=ds[:])
                    nc.vector.reciprocal(rs[:], ds[:])
                elif path == "p":
                    v = wpool.tile([128, N], BF16, tag="pv",
                                   name=f"pv{h}_{js}")
                    nc.gpsimd.scalar_tensor_tensor(
                        out=v[:], in0=e1rb[h][:],
                        scalar=e4c_sb[js][:, h:h + 1], in1=e5rb[h][:],
                        op0=M.mult, op1=M.max)
                    nc.gpsimd.scalar_tensor_tensor(
                        out=et[:], in0=v[:], scalar=1.0, in1=at_sb[js][:],
                        op0=M.mult, op1=M.mult, accum_out=ds[:])
                    nc.vector.reciprocal(rs[:], ds[:])
                else:  # 'a' — ACT relu path (uncanceled convention)
                    craw = craw_sb[js][:, h:h + 1]
                    br = spool.tile([128, 1], F32, tag="br",
                                    name=f"br{h}_{js}")
                    nc.vector.tensor_scalar_add(br[:], craw, -200.0)
                    be = spool.tile([128, 1], F32, tag="be",
                                    name=f"be{h}_{js}")
                    nc.vector.tensor_scalar(
                        out=be[:], in0=craw, scalar1=0.2, scalar2=-40.0,
                        op0=M.mult, op1=M.add)
                    zt = ps1.tile([128, N], F32, tag="srow",
                                  name=f"z{h}_{js}")
                    for ns in (slice(0, 512), slice(512, 1024)):
                        nc.tensor.matmul(zt[:, ns], eye[:, 0:128],
                                         at_sb[js][:, ns],
                                         start=True, stop=False)
                        nc.tensor.matmul(zt[:, ns], ones1[0:1, :],
                                         srb[h][0:1, ns],
                                         start=False, stop=True)
                    rt = wpool.tile([128, N], BF16, tag="rt",
                                    name=f"rt{h}_{js}")
                    nc.scalar.activation(rt[:], zt[:], AF.Relu,
                                         bias=br[:], scale=5.0)
                    zt2 = ps1.tile([128, N], F32, tag="srow",
                                   name=f"z2_{h}_{js}")
                    for ns in (slice(0, 512), slice(512, 1024)):
                        nc.tensor.matmul(zt2[:, ns], eye[:, 0:128],
                                         at_sb[js][:, ns],
                                         start=True, stop=False)
                        nc.tensor.matmul(zt2[:, ns], ones1[0:1, :],
                                         srb[h][0:1, ns],
                                         start=False, stop=False)
                        nc.tensor.matmul(zt2[:, ns], eye[:, 128:256],
                                         rt[:, ns], start=False, stop=True)
                    nc.scalar.activation(et[:], zt2[:], AF.Exp,
                                         bias=be[:], scale=1.0,
                                         accum_out=ds[:])
                    nc.vector.reciprocal(rs[:], ds[:])
                whp = etpool.tile([128, F_OUT], BF16, tag=f"whp{h}_{js}",
                                  name=f"whp{h}_{js}")
                nc.vector.tensor_scalar(out=whp[:],
                                        in0=wh_sb[js][:, h * F_OUT:(h + 1) * F_OUT],
                                        scalar1=rs[:], scalar2=None,
                                        op0=M.mult)
                et_sb[(h, js)] = et
                whp_sb[(h, js)] = whp

            def emit_mm(h, js, start, stop):
                pair, po = h // 2, (h % 2) * 64
                tp = (0, po) if po else None
                for nch in range(2):
                    ns = slice(nch * 512, (nch + 1) * 512)
                    nc.tensor.matmul(ot_ps[pair][po:po + 64, ns],
                                     whp_sb[(h, js)][:],
                                     et_sb[(h, js)][:, ns],
                                     start=start, stop=stop,
                                     tile_position=tp)

            # Fields are emitted js-major in two waves, balanced across
            # engines; ET matmuls follow estimated field-completion order,
            # subject to: all h0 MMs before any h1 MM, all h2 before any h3
            # (heads sharing a PSUM pair must run their accumulation groups
            # sequentially).  Within a head any js order is legal; start/stop
            # flags follow emission order.
            # Estimated completion time per strip, by engine slot.
            est = {}
            cost = {"d": 1.31, "p": 1.9, "a": 2.8}
            vclock = {"d": 4.9, "p": 4.8, "a": 16.0}

            def note(h, js, path):
                vclock[path] += cost[path]
                est[(h, js)] = vclock[path]

            # Wave A: fields of h0 (DVE), h2 (Pool), h3 a-strips (ACT/PE);
            # ET-MMs of h0 and h2 interleaved js-major.  Wave-B broadcasts
            # slot in after the first strips.
            for js in range(NSTRIP):
                emit_field(0, js, PATHS[0][js])
                note(0, js, PATHS[0][js])
                emit_field(2, js, PATHS[2][js])
                note(2, js, PATHS[2][js])
                if PATHS[3][js] == "a":
                    emit_field(3, js, "a")
                    note(3, js, "a")
                if js == 1:
                    emit_bcast(1)
                    vclock["p"] += 1.9  # two broadcasts occupy Pool
                elif js == 2:
                    emit_bcast(3)
                    vclock["p"] += 0.95
                emit_mm(0, js, js == 0, js == NSTRIP - 1)
                emit_mm(2, js, js == 0, js == NSTRIP - 1)
            # Wave B fields
            for js in range(NSTRIP):
                emit_field(1, js, PATHS[1][js])
                note(1, js, PATHS[1][js])
                if PATHS[3][js] != "a":
                    emit_field(3, js, PATHS[3][js])
                    note(3, js, PATHS[3][js])
            # Wave B ET-MMs in estimated ready order: a head's MMs cannot
            # execute before the preceding head's group (same PSUM pair)
            # has closed, so clamp by that head's last field time.
            gate = {1: max(est[(0, js)] for js in range(NSTRIP)) + 0.2,
                    3: max(est[(2, js)] for js in range(NSTRIP)) + 0.2}
            mmlist = sorted(
                [(max(est[(h, js)], gate[h]), h, js)
                 for h in (1, 3) for js in range(NSTRIP)])
            seen = {1: 0, 3: 0}
            for _, h, js in mmlist:
                seen[h] += 1
                emit_mm(h, js, seen[h] == 1, seen[h] == NSTRIP)

            # ---- phase 4: write out (bf16, chunked; 3 DMA queues) --------
            dma_engs = [nc.sync, nc.gpsimd, nc.scalar]
            di = 0
            for pair in range(2):
                ot_sb = cpool.tile([128, N], BF16, tag=f"otsb{pair}",
                                   name=f"otsb{pair}")
                for nch in range(2):
                    ns = slice(nch * 512, (nch + 1) * 512)
                    if nch % 2 == 0:
                        nc.scalar.copy(ot_sb[:, ns], ot_ps[pair][:, ns])
                    else:
                        nc.vector.tensor_copy(ot_sb[:, ns], ot_ps[pair][:, ns])
                    for hh in range(2):
                        h, po = pair * 2 + hh, hh * 64
                        eng = dma_engs[di % 3]
                        di += 1
                        eng.dma_start(ot_d[h][:, ns], ot_sb[po:po + 64, ns])

    nc.compile()
    return nc


# --------------------------------------------------------------------------
# host-side pre/post processing
# --------------------------------------------------------------------------
def prep_in_maps(A, x, W, a_w, a_b):
    A = np.asarray(A, np.float32)
    x = np.asarray(x, np.float32)
    W = np.asarray(W, np.float32)
    a_w = np.asarray(a_w, np.float32)
    a_b = np.asarray(a_b, np.float32)

    a1, a2 = a_w[:, :F_OUT], a_w[:, F_OUT:]
    w1c = np.einsum("hfo,ho->fh", W, a1).astype(np.float32)  # W@a1 full scale
    w1s = np.zeros((F_IN, 113), np.float32)
    w1s[:, 0::32] = NEG_SLOPE * w1c
    w1s[:, 16::32] = w1c
    w1s = w1s.astype(nbf16)
    w2raw = np.einsum("hfo,ho->fh", W, a2).astype(np.float32)
    w8s = 0.8 * w2raw
    w4 = W.transpose(1, 0, 2).reshape(F_IN, H * F_OUT)  # [f, h*F_OUT+o]
    wcat = np.concatenate([w4, w8s, w2raw], axis=1).astype(nbf16)
    bcol = np.zeros((128, 1), np.float32)
    bcol[0::32, 0][:H] = NEG_SLOPE * a_b
    bcol[16::32, 0][:H] = a_b
    bline = np.concatenate([a_b, NEG_SLOPE * a_b]).reshape(1, 2 * H)
    bline = bline.astype(np.float32)
    eye_np = np.concatenate([40.0 * np.eye(128, dtype=np.float32),
                             0.8 * np.eye(128, dtype=np.float32)],
                            axis=1).astype(nbf16)

    at_c = [np.ascontiguousarray(A[c].T).astype(nbf16) for c in range(NCORES)]

    in_maps = []
    for c in range(NCORES):
        in_maps.append({
            "at": at_c[c],
            "xt": np.ascontiguousarray(x[c].T).astype(nbf16),
            "wcat": wcat,
            "w1s": w1s,
            "bcol": bcol,
            "bline": bline,
            "eye": eye_np,
            "ones1": np.ones((1, 128), np.float32).astype(nbf16),
        })
    return in_maps


def postprocess(results):
    out = np.empty((B, N, H * F_OUT), np.float32)
    for c in range(NCORES):
        ot = np.asarray(results[c]["ot"], np.float32)  # [H, F_OUT, N] bf16
        out[c] = ot.transpose(2, 0, 1).reshape(N, H * F_OUT)
    return out


_NC_CACHE = None


def get_nc():
    global _NC_CACHE
    if _NC_CACHE is None:
        _NC_CACHE = build_nc()
    return _NC_CACHE


def kernel(A, x, W, a_w, a_b):
    nc = get_nc()
    in_maps = prep_in_maps(A, x, W, a_w, a_b)
    res = run_bass_kernel_spmd(nc, in_maps, core_ids=list(range(NCORES)))
    return postprocess(res.results)
